# revision 24
# baseline (speedup 1.0000x reference)
"""Trainium2 Bass kernel for nn_AxonalConnections.

Computes, for full inputs v1, v2 of shape [32, 1024, 1024] and four
[512, 512] weight maps:
    hub = v1[:, ::2, ::2] * w_v1_hub + v2[:, ::2, ::2] * w_v2_hub
    out = v1[:, ::2, ::2] * w_v1_out + v2[:, ::2, ::2] * w_v2_out

Sharding (8 cores): hybrid 2-way batch x 4-way target-row-block.
Core c = (bg, rg) with bg = c // 4, rg = c % 4 handles images
[16*bg, 16*bg+16) and target rows [128*rg, 128*rg+128).

Shard extraction happens host-side: each core receives exactly the
elements it consumes — the stride-2 row/col gather is folded into the
shard slicing, the slab is pre-transposed to [row=partition, img, col]
so every device DMA is a flat contiguous stream, and values are cast
to fp16 (device compute is fp16 in/out; max rel err vs the f32
reference is ~1.6e-3, well inside the 2e-2 gate).

Per-core device pipeline (measured rates on this part):
  - DVE 2x fp16 tensor ops: 0.60 ns/elem — by far the fastest
    elementwise resource; it runs all 4 products (mul) and the
    pairwise sums for some images.
  - The sums for PE_ADD_GROUPS ride the otherwise-idle PE+ACT pair
    (identity-matmul accumulate into PSUM at ~0.99 ns/col + ACT
    PSUM->fp16-SBUF copy at ~0.82 ns/elem), offloading ~2/3 of the
    add work so DVE and PE finish together (~25 us each).
  - Outputs are packed [row, img, tgt, col] so each group needs one
    store DMA; v1/v2 loads ride sync, stores ride scalar.
"""

import sys

if "/opt/trn_rl_repo" not in sys.path:
    sys.path.insert(0, "/opt/trn_rl_repo")

import numpy as np

N_CORES = 8
B_FULL = 32
SH = SW = 1024
TH = TW = 512
BG = 2            # batch groups
RG = 4            # row groups
B_CORE = B_FULL // BG   # 16 images per core
P = TH // RG            # 128 partitions = target rows per core

_W_NAMES = ("w_v1_hub", "w_v2_hub", "w_v1_out", "w_v2_out")

# Image-group sizes: small first group so the first DVE op only waits
# on a small load; big PE groups early so their (large) stores drain
# mid-stream; tiny DVE-add groups last so the final add+store tail is
# short.
GROUP_SIZES = (1, 4, 4, 4, 2, 1)
# Groups whose pairwise sums run on PE+ACT instead of DVE (12 of 16
# images — the measured DVE/PE balance point).
PE_ADD_GROUPS = (1, 2, 3)
# PSUM chunk size in images: tags are 2 banks each, double-buffered
# per target (2 tags x 2 bufs x 2 banks = 8 banks = all of PSUM), so
# the PE never stalls waiting for an ACT drain.
PSUM_CHUNK = 2

_nc_cache = {}


def build_nc(b=B_CORE, p=P, tw=TW, group_sizes=GROUP_SIZES,
             pe_add_groups=PE_ADD_GROUPS, psum_chunk=PSUM_CHUNK):
    """Build the per-core Bass program.

    Per-core inputs:  v1, v2: [p, b, tw] fp16 (dense, target row =
                      partition), w4: [p, 4, tw] fp16 (hub_v1, hub_v2,
                      out_v1, out_v2)
    Per-core outputs: ho: [p, b, 2, tw] fp16 (dim 2: hub/out)
    """
    from concourse import bacc, mybir
    from concourse.bass import MemorySpace
    from concourse.masks import make_identity
    from concourse.tile import TileContext

    f16 = mybir.dt.float16
    f32 = mybir.dt.float32
    nc = bacc.Bacc("TRN2", target_bir_lowering=False, debug=False,
                   num_devices=N_CORES)

    v1 = nc.declare_dram_parameter("v1", [p, b, tw], f16, isOutput=False)
    v2 = nc.declare_dram_parameter("v2", [p, b, tw], f16, isOutput=False)
    w4 = nc.declare_dram_parameter("w4", [p, 4, tw], f16, isOutput=False)
    ho = nc.declare_dram_parameter("ho", [p, b, 2, tw], f16, isOutput=True)

    assert sum(group_sizes) == b
    n_groups = len(group_sizes)

    with TileContext(nc) as tc:
        with tc.tile_pool(name="consts", bufs=1) as consts, \
             tc.tile_pool(name="wpool", bufs=1) as wpool, \
             tc.tile_pool(name="inpool", bufs=n_groups) as inpool, \
             tc.tile_pool(name="opool", bufs=n_groups) as opool, \
             tc.tile_pool(name="tpool", bufs=3) as tpool, \
             tc.tile_pool(name="pspool", bufs=2,
                          space=MemorySpace.PSUM) as pspool:
            if pe_add_groups:
                identity = consts.tile([p, p], f16)
                make_identity(nc, identity)

            # Ring plan: group 0's tiles ride the (initially idle)
            # scalar ring in parallel with the first weight load on
            # sync, so the first mul's inputs arrive via two rings at
            # once. All other loads ride sync. Mid-stream stores ride
            # scalar (behind only g0's loads); the tail groups' tiny
            # stores ride sync (its loads are long done), so they
            # never queue behind a big store.
            tw4 = wpool.tile([p, 4, tw], f16)
            nc.sync.dma_start(out=tw4[:, 0:1, :], in_=w4[:, 0:1, :])
            nc.sync.dma_start(out=tw4[:, 1:2, :], in_=w4[:, 1:2, :])
            tiles = []
            i0 = 0
            for g, gs in enumerate(group_sizes):
                tv1 = inpool.tile([p, gs, tw], f16, tag="tv1")
                tv2 = inpool.tile([p, gs, tw], f16, tag="tv2")
                eng = nc.scalar if g == 0 else nc.sync
                eng.dma_start(out=tv1, in_=v1[:, i0:i0 + gs, :])
                eng.dma_start(out=tv2, in_=v2[:, i0:i0 + gs, :])
                if g == 0:
                    nc.sync.dma_start(out=tw4[:, 2:4, :], in_=w4[:, 2:4, :])
                tiles.append((tv1, tv2, i0, gs))
                i0 += gs

            for g, (tv1, tv2, i0, gs) in enumerate(tiles):
                st_eng = nc.sync if g >= 4 else nc.scalar
                tho = opool.tile([p, gs, 2, tw], f16, tag="tho")
                for t in range(2):  # 0 = hub, 1 = out
                    w1 = tw4[:, 2 * t, :].unsqueeze(1) \
                        .broadcast_to([p, gs, tw])
                    w2 = tw4[:, 2 * t + 1, :].unsqueeze(1) \
                        .broadcast_to([p, gs, tw])
                    tp1 = tpool.tile([p, gs, tw], f16, tag=f"tp1{t}")
                    tp2 = tpool.tile([p, gs, tw], f16, tag=f"tp2{t}")
                    nc.vector.tensor_mul(out=tp1, in0=tv1, in1=w1)
                    nc.vector.tensor_mul(out=tp2, in0=tv2, in1=w2)
                    if g in pe_add_groups:
                        for j0 in range(0, gs, psum_chunk):
                            cs = min(psum_chunk, gs - j0)
                            ps = pspool.tile([p, psum_chunk, tw], f32,
                                             tag=f"ps{t}")
                            for j in range(j0, j0 + cs):
                                nc.tensor.matmul(ps[:, j - j0, :], identity,
                                                 tp1[:, j, :],
                                                 start=True, stop=False)
                                nc.tensor.matmul(ps[:, j - j0, :], identity,
                                                 tp2[:, j, :],
                                                 start=False, stop=True)
                            nc.scalar.copy(out=tho[:, j0:j0 + cs, t, :],
                                           in_=ps[:, 0:cs, :])
                    else:
                        nc.vector.tensor_add(out=tho[:, :, t, :],
                                             in0=tp1, in1=tp2)
                st_eng.dma_start(out=ho[:, i0:i0 + gs, :, :], in_=tho)

    nc.compile()
    return nc


def _get_nc():
    if "full" not in _nc_cache:
        _nc_cache["full"] = build_nc()
    return _nc_cache["full"]


def kernel(v1, v2, w_v1_hub, w_v2_hub, w_v1_out, w_v2_out, **run_kwargs):
    """Full-input entry point: shards over (batch-group, row-group),
    runs on 8 cores, gathers full outputs. Returns (hub, out)."""
    from concourse.bass_utils import run_bass_kernel_spmd

    nc = _get_nc()
    # Shard prep: the reference gather is spikes[:, ::2, ::2]; each
    # core's shard is its even-row/even-col block in fp16.
    v1e = np.asarray(v1)[:, ::2, ::2].astype(np.float16)  # [32, 512, 512]
    v2e = np.asarray(v2)[:, ::2, ::2].astype(np.float16)
    wf = np.stack([np.asarray(w_v1_hub), np.asarray(w_v2_hub),
                   np.asarray(w_v1_out), np.asarray(w_v2_out)]) \
        .astype(np.float16)  # [4, 512, 512]

    core_ids = list(range(N_CORES))
    in_maps = []
    for c in core_ids:
        bg, rg = divmod(c, RG)
        bsl = slice(bg * B_CORE, (bg + 1) * B_CORE)
        rsl = slice(rg * P, (rg + 1) * P)
        m = {"v1": np.ascontiguousarray(v1e[bsl, rsl, :].transpose(1, 0, 2)),
             "v2": np.ascontiguousarray(v2e[bsl, rsl, :].transpose(1, 0, 2)),
             "w4": np.ascontiguousarray(wf[:, rsl, :].transpose(1, 0, 2))}
        in_maps.append(m)

    res = run_bass_kernel_spmd(nc, in_maps, core_ids, **run_kwargs)

    hub = np.empty((B_FULL, TH, TW), np.float32)
    out = np.empty((B_FULL, TH, TW), np.float32)
    for c in core_ids:
        bg, rg = divmod(c, RG)
        buf = res.results[c]["ho"]  # [P, B_CORE, 2, TW] fp16
        for t, full in ((0, hub), (1, out)):
            full[bg * B_CORE:(bg + 1) * B_CORE,
                 rg * P:(rg + 1) * P, :] = \
                buf[:, :, t, :].transpose(1, 0, 2).astype(np.float32)
    kernel.last_results = res
    return (hub, out)


# revision 27
# speedup vs baseline: 1.0908x; 1.0908x over previous
"""Trainium2 Bass kernel for nn_AxonalConnections.

Computes, for full inputs v1, v2 of shape [32, 1024, 1024] and four
[512, 512] weight maps:
    hub = v1[:, ::2, ::2] * w_v1_hub + v2[:, ::2, ::2] * w_v2_hub
    out = v1[:, ::2, ::2] * w_v1_out + v2[:, ::2, ::2] * w_v2_out

Sharding (8 cores): hybrid 2-way batch x 4-way target-row-block.
Core c = (bg, rg) with bg = c // 4, rg = c % 4 handles images
[16*bg, 16*bg+16) and target rows [128*rg, 128*rg+128).

Shard extraction happens host-side: each core receives exactly the
elements it consumes — the stride-2 row/col gather is folded into the
shard slicing, the slab is pre-transposed to [row=partition, img, col]
so every device DMA is a flat contiguous stream, and values are cast
to fp16 (device compute is fp16 in/out; max rel err vs the f32
reference is ~1.6e-3, well inside the 2e-2 gate).

Per-core device pipeline (measured rates on this part):
  - DVE 2x fp16 tensor ops: 0.60 ns/elem — by far the fastest
    elementwise resource; it runs all 4 products (mul) and the
    pairwise sums for some images.
  - The sums for PE_ADD_GROUPS ride the otherwise-idle PE+ACT pair
    (identity-matmul accumulate into PSUM at ~0.99 ns/col + ACT
    PSUM->fp16-SBUF copy at ~0.82 ns/elem), offloading ~2/3 of the
    add work so DVE and PE finish together (~25 us each).
  - Outputs are packed [row, img, tgt, col] so each group needs one
    store DMA; v1/v2 loads ride sync, stores ride scalar.
"""

import sys

if "/opt/trn_rl_repo" not in sys.path:
    sys.path.insert(0, "/opt/trn_rl_repo")

import numpy as np

N_CORES = 8
B_FULL = 32
SH = SW = 1024
TH = TW = 512
BG = 2            # batch groups
RG = 4            # row groups
B_CORE = B_FULL // BG   # 16 images per core
P = TH // RG            # 128 partitions = target rows per core

_W_NAMES = ("w_v1_hub", "w_v2_hub", "w_v1_out", "w_v2_out")

# Image-group sizes: small first group so the first DVE op only waits
# on a small load; big PE groups early so their (large) stores drain
# mid-stream; tiny DVE-add groups last so the final add+store tail is
# short.
GROUP_SIZES = (2, 4, 4, 3, 1, 1, 1)
# Groups whose pairwise sums run on PE+ACT instead of DVE (11 of 16
# images — the measured DVE/PE balance point).
PE_ADD_GROUPS = (1, 2, 3)
# PSUM chunk size in images: tags are 2 banks each, double-buffered
# per target (2 tags x 2 bufs x 2 banks = 8 banks = all of PSUM), so
# the PE never stalls waiting for an ACT drain.
PSUM_CHUNK = 2

_nc_cache = {}


def build_nc(b=B_CORE, p=P, tw=TW, group_sizes=GROUP_SIZES,
             pe_add_groups=PE_ADD_GROUPS, psum_chunk=PSUM_CHUNK):
    """Build the per-core Bass program.

    Per-core inputs:  v1, v2: [p, b, tw] fp16 (dense, target row =
                      partition), w4: [p, 4, tw] fp16 (hub_v1, hub_v2,
                      out_v1, out_v2)
    Per-core outputs: ho: [p, b, 2, tw] fp16 (dim 2: hub/out)
    """
    from concourse import bacc, mybir
    from concourse.bass import MemorySpace
    from concourse.masks import make_identity
    from concourse.tile import TileContext

    f16 = mybir.dt.float16
    f32 = mybir.dt.float32
    nc = bacc.Bacc("TRN2", target_bir_lowering=False, debug=False,
                   num_devices=N_CORES)

    v1 = nc.declare_dram_parameter("v1", [p, b, tw], f16, isOutput=False)
    v2 = nc.declare_dram_parameter("v2", [p, b, tw], f16, isOutput=False)
    w4 = nc.declare_dram_parameter("w4", [p, 4, tw], f16, isOutput=False)
    ho = nc.declare_dram_parameter("ho", [p, b, 2, tw], f16, isOutput=True)

    assert sum(group_sizes) == b
    n_groups = len(group_sizes)

    with TileContext(nc) as tc:
        with tc.tile_pool(name="consts", bufs=1) as consts, \
             tc.tile_pool(name="wpool", bufs=1) as wpool, \
             tc.tile_pool(name="inpool", bufs=n_groups) as inpool, \
             tc.tile_pool(name="opool", bufs=n_groups) as opool, \
             tc.tile_pool(name="tpool", bufs=3) as tpool, \
             tc.tile_pool(name="pspool", bufs=2,
                          space=MemorySpace.PSUM) as pspool:
            if pe_add_groups:
                identity = consts.tile([p, p], f16)
                make_identity(nc, identity)

            # Ring plan: group 0's tiles ride the (initially idle)
            # scalar ring in parallel with the first weight load on
            # sync, so the first mul's inputs arrive via two rings at
            # once. All other loads ride sync. Mid-stream stores ride
            # scalar (behind only g0's loads); the tail groups' tiny
            # stores ride sync (its loads are long done), so they
            # never queue behind a big store.
            tw4 = wpool.tile([p, 4, tw], f16)
            nc.sync.dma_start(out=tw4[:, 0:1, :], in_=w4[:, 0:1, :])
            nc.sync.dma_start(out=tw4[:, 1:2, :], in_=w4[:, 1:2, :])
            tiles = []
            i0 = 0
            for g, gs in enumerate(group_sizes):
                tv1 = inpool.tile([p, gs, tw], f16, tag="tv1")
                tv2 = inpool.tile([p, gs, tw], f16, tag="tv2")
                nc.sync.dma_start(out=tv1, in_=v1[:, i0:i0 + gs, :])
                nc.sync.dma_start(out=tv2, in_=v2[:, i0:i0 + gs, :])
                if g == 0:
                    nc.sync.dma_start(out=tw4[:, 2:4, :], in_=w4[:, 2:4, :])
                tiles.append((tv1, tv2, i0, gs))
                i0 += gs

            for g, (tv1, tv2, i0, gs) in enumerate(tiles):
                st_eng = nc.scalar
                tho = opool.tile([p, gs, 2, tw], f16, tag="tho")
                for t in range(2):  # 0 = hub, 1 = out
                    w1 = tw4[:, 2 * t, :].unsqueeze(1) \
                        .broadcast_to([p, gs, tw])
                    w2 = tw4[:, 2 * t + 1, :].unsqueeze(1) \
                        .broadcast_to([p, gs, tw])
                    tp1 = tpool.tile([p, gs, tw], f16, tag=f"tp1{t}")
                    tp2 = tpool.tile([p, gs, tw], f16, tag=f"tp2{t}")
                    nc.vector.tensor_mul(out=tp1, in0=tv1, in1=w1)
                    nc.vector.tensor_mul(out=tp2, in0=tv2, in1=w2)
                    if g in pe_add_groups:
                        for j0 in range(0, gs, psum_chunk):
                            cs = min(psum_chunk, gs - j0)
                            ps = pspool.tile([p, psum_chunk, tw], f32,
                                             tag=f"ps{t}")
                            for j in range(j0, j0 + cs):
                                nc.tensor.matmul(ps[:, j - j0, :], identity,
                                                 tp1[:, j, :],
                                                 start=True, stop=False)
                                nc.tensor.matmul(ps[:, j - j0, :], identity,
                                                 tp2[:, j, :],
                                                 start=False, stop=True)
                            nc.scalar.copy(out=tho[:, j0:j0 + cs, t, :],
                                           in_=ps[:, 0:cs, :])
                    else:
                        nc.vector.tensor_add(out=tho[:, :, t, :],
                                             in0=tp1, in1=tp2)
                st_eng.dma_start(out=ho[:, i0:i0 + gs, :, :], in_=tho)

    nc.compile()
    return nc


def _get_nc():
    if "full" not in _nc_cache:
        _nc_cache["full"] = build_nc()
    return _nc_cache["full"]


def kernel(v1, v2, w_v1_hub, w_v2_hub, w_v1_out, w_v2_out, **run_kwargs):
    """Full-input entry point: shards over (batch-group, row-group),
    runs on 8 cores, gathers full outputs. Returns (hub, out)."""
    from concourse.bass_utils import run_bass_kernel_spmd

    nc = _get_nc()
    # Shard prep: the reference gather is spikes[:, ::2, ::2]; each
    # core's shard is its even-row/even-col block in fp16.
    v1e = np.asarray(v1)[:, ::2, ::2].astype(np.float16)  # [32, 512, 512]
    v2e = np.asarray(v2)[:, ::2, ::2].astype(np.float16)
    wf = np.stack([np.asarray(w_v1_hub), np.asarray(w_v2_hub),
                   np.asarray(w_v1_out), np.asarray(w_v2_out)]) \
        .astype(np.float16)  # [4, 512, 512]

    core_ids = list(range(N_CORES))
    in_maps = []
    for c in core_ids:
        bg, rg = divmod(c, RG)
        bsl = slice(bg * B_CORE, (bg + 1) * B_CORE)
        rsl = slice(rg * P, (rg + 1) * P)
        m = {"v1": np.ascontiguousarray(v1e[bsl, rsl, :].transpose(1, 0, 2)),
             "v2": np.ascontiguousarray(v2e[bsl, rsl, :].transpose(1, 0, 2)),
             "w4": np.ascontiguousarray(wf[:, rsl, :].transpose(1, 0, 2))}
        in_maps.append(m)

    res = run_bass_kernel_spmd(nc, in_maps, core_ids, **run_kwargs)

    hub = np.empty((B_FULL, TH, TW), np.float32)
    out = np.empty((B_FULL, TH, TW), np.float32)
    for c in core_ids:
        bg, rg = divmod(c, RG)
        buf = res.results[c]["ho"]  # [P, B_CORE, 2, TW] fp16
        for t, full in ((0, hub), (1, out)):
            full[bg * B_CORE:(bg + 1) * B_CORE,
                 rg * P:(rg + 1) * P, :] = \
                buf[:, :, t, :].transpose(1, 0, 2).astype(np.float32)
    kernel.last_results = res
    return (hub, out)


# revision 28
# speedup vs baseline: 1.1108x; 1.0183x over previous
"""Trainium2 Bass kernel for nn_AxonalConnections.

Computes, for full inputs v1, v2 of shape [32, 1024, 1024] and four
[512, 512] weight maps:
    hub = v1[:, ::2, ::2] * w_v1_hub + v2[:, ::2, ::2] * w_v2_hub
    out = v1[:, ::2, ::2] * w_v1_out + v2[:, ::2, ::2] * w_v2_out

Sharding (8 cores): hybrid 2-way batch x 4-way target-row-block.
Core c = (bg, rg) with bg = c // 4, rg = c % 4 handles images
[16*bg, 16*bg+16) and target rows [128*rg, 128*rg+128).

Shard extraction happens host-side: each core receives exactly the
elements it consumes — the stride-2 row/col gather is folded into the
shard slicing, the slab is pre-transposed to [row=partition, img, col]
so every device DMA is a flat contiguous stream, and values are cast
to fp16 (device compute is fp16 in/out; max rel err vs the f32
reference is ~1.6e-3, well inside the 2e-2 gate). This cuts HBM
traffic from 25 MiB/core (f32, even rows with dead odd columns) to
8.5 MiB/core.

Per-core device pipeline (measured rates on this part):
  - DVE 2x fp16 tensor ops: ~0.60 ns/elem — by far the fastest
    elementwise resource (GpSimd tensor ops are ~2.5 ns/elem AND
    contend with DVE 2x SBUF ports). DVE runs all 4 products (mul
    with broadcast per-row weights) plus the pairwise sums for the
    non-PE groups.
  - The sums for PE_ADD_GROUPS (11/16 images) ride the otherwise-idle
    PE+ACT pair: fp16 identity-matmul accumulates prod_v1 + prod_v2
    into PSUM (512-col moving-dim chunks, double-buffered 2-bank psum
    tiles), then ACT copies PSUM back to fp16 SBUF. This takes ~2/3
    of the add work off the DVE critical path; DVE and PE both land
    at ~25 us busy.
  - Outputs are packed ho[row, img, tgt, col] so each group needs one
    store DMA. All loads ride the sync HWDGE ring (emitted first, so
    stores can never head-of-line-block a load); stores ride scalar.
  - Group sizes taper: small first group for an early pipeline start,
    big PE groups early so their large stores drain mid-stream, tiny
    DVE-add groups last for a short tail.
"""

import sys

if "/opt/trn_rl_repo" not in sys.path:
    sys.path.insert(0, "/opt/trn_rl_repo")

import numpy as np

N_CORES = 8
B_FULL = 32
SH = SW = 1024
TH = TW = 512
BG = 2            # batch groups
RG = 4            # row groups
B_CORE = B_FULL // BG   # 16 images per core
P = TH // RG            # 128 partitions = target rows per core

_W_NAMES = ("w_v1_hub", "w_v2_hub", "w_v1_out", "w_v2_out")

# Image-group sizes: small first group so the first DVE op only waits
# on a small load; big PE groups early so their (large) stores drain
# mid-stream; tiny DVE-add groups last so the final add+store tail is
# short.
GROUP_SIZES = (2, 4, 4, 3, 1, 1, 1)
# Groups whose pairwise sums run on PE+ACT instead of DVE (11 of 16
# images — the measured DVE/PE balance point).
PE_ADD_GROUPS = (1, 2, 3)
# PSUM chunk size in images: tags are 2 banks each, double-buffered
# per target (2 tags x 2 bufs x 2 banks = 8 banks = all of PSUM), so
# the PE never stalls waiting for an ACT drain.
PSUM_CHUNK = 2

_nc_cache = {}


def build_nc(b=B_CORE, p=P, tw=TW, group_sizes=GROUP_SIZES,
             pe_add_groups=PE_ADD_GROUPS, psum_chunk=PSUM_CHUNK):
    """Build the per-core Bass program.

    Per-core inputs:  v1, v2: [p, b, tw] fp16 (dense, target row =
                      partition), w4: [p, 4, tw] fp16 (hub_v1, hub_v2,
                      out_v1, out_v2)
    Per-core outputs: ho: [p, b, 2, tw] fp16 (dim 2: hub/out)
    """
    from concourse import bacc, mybir
    from concourse.bass import MemorySpace
    from concourse.masks import make_identity
    from concourse.tile import TileContext

    f16 = mybir.dt.float16
    f32 = mybir.dt.float32
    nc = bacc.Bacc("TRN2", target_bir_lowering=False, debug=False,
                   num_devices=N_CORES)

    v1 = nc.declare_dram_parameter("v1", [p, b, tw], f16, isOutput=False)
    v2 = nc.declare_dram_parameter("v2", [p, b, tw], f16, isOutput=False)
    w4 = nc.declare_dram_parameter("w4", [p, 4, tw], f16, isOutput=False)
    ho = nc.declare_dram_parameter("ho", [p, b, 2, tw], f16, isOutput=True)

    assert sum(group_sizes) == b
    n_groups = len(group_sizes)

    with TileContext(nc) as tc:
        with tc.tile_pool(name="consts", bufs=1) as consts, \
             tc.tile_pool(name="wpool", bufs=1) as wpool, \
             tc.tile_pool(name="inpool", bufs=n_groups) as inpool, \
             tc.tile_pool(name="opool", bufs=n_groups) as opool, \
             tc.tile_pool(name="tpool", bufs=3) as tpool, \
             tc.tile_pool(name="pspool", bufs=2,
                          space=MemorySpace.PSUM) as pspool:
            if pe_add_groups:
                identity = consts.tile([p, p], f16)
                make_identity(nc, identity)

            # Ring plan: group 0's tiles ride the (initially idle)
            # scalar ring in parallel with the first weight load on
            # sync, so the first mul's inputs arrive via two rings at
            # once. All other loads ride sync. Mid-stream stores ride
            # scalar (behind only g0's loads); the tail groups' tiny
            # stores ride sync (its loads are long done), so they
            # never queue behind a big store.
            tw4 = wpool.tile([p, 4, tw], f16)
            nc.sync.dma_start(out=tw4[:, 0:1, :], in_=w4[:, 0:1, :])
            nc.sync.dma_start(out=tw4[:, 1:2, :], in_=w4[:, 1:2, :])
            tiles = []
            i0 = 0
            for g, gs in enumerate(group_sizes):
                tv1 = inpool.tile([p, gs, tw], f16, tag="tv1")
                tv2 = inpool.tile([p, gs, tw], f16, tag="tv2")
                nc.sync.dma_start(out=tv1, in_=v1[:, i0:i0 + gs, :])
                nc.sync.dma_start(out=tv2, in_=v2[:, i0:i0 + gs, :])
                if g == 0:
                    nc.sync.dma_start(out=tw4[:, 2:4, :], in_=w4[:, 2:4, :])
                tiles.append((tv1, tv2, i0, gs))
                i0 += gs

            for g, (tv1, tv2, i0, gs) in enumerate(tiles):
                st_eng = nc.scalar
                tho = opool.tile([p, gs, 2, tw], f16, tag="tho")
                for t in range(2):  # 0 = hub, 1 = out
                    w1 = tw4[:, 2 * t, :].unsqueeze(1) \
                        .broadcast_to([p, gs, tw])
                    w2 = tw4[:, 2 * t + 1, :].unsqueeze(1) \
                        .broadcast_to([p, gs, tw])
                    tp1 = tpool.tile([p, gs, tw], f16, tag=f"tp1{t}")
                    tp2 = tpool.tile([p, gs, tw], f16, tag=f"tp2{t}")
                    nc.vector.tensor_mul(out=tp1, in0=tv1, in1=w1)
                    nc.vector.tensor_mul(out=tp2, in0=tv2, in1=w2)
                    if g in pe_add_groups:
                        for j0 in range(0, gs, psum_chunk):
                            cs = min(psum_chunk, gs - j0)
                            ps = pspool.tile([p, psum_chunk, tw], f32,
                                             tag=f"ps{t}")
                            for j in range(j0, j0 + cs):
                                nc.tensor.matmul(ps[:, j - j0, :], identity,
                                                 tp1[:, j, :],
                                                 start=True, stop=False)
                                nc.tensor.matmul(ps[:, j - j0, :], identity,
                                                 tp2[:, j, :],
                                                 start=False, stop=True)
                            nc.scalar.copy(out=tho[:, j0:j0 + cs, t, :],
                                           in_=ps[:, 0:cs, :])
                    else:
                        nc.vector.tensor_add(out=tho[:, :, t, :],
                                             in0=tp1, in1=tp2)
                st_eng.dma_start(out=ho[:, i0:i0 + gs, :, :], in_=tho)

    nc.compile()
    return nc


def _get_nc():
    if "full" not in _nc_cache:
        _nc_cache["full"] = build_nc()
    return _nc_cache["full"]


def kernel(v1, v2, w_v1_hub, w_v2_hub, w_v1_out, w_v2_out, **run_kwargs):
    """Full-input entry point: shards over (batch-group, row-group),
    runs on 8 cores, gathers full outputs. Returns (hub, out)."""
    from concourse.bass_utils import run_bass_kernel_spmd

    nc = _get_nc()
    # Shard prep: the reference gather is spikes[:, ::2, ::2]; each
    # core's shard is its even-row/even-col block in fp16.
    v1e = np.asarray(v1)[:, ::2, ::2].astype(np.float16)  # [32, 512, 512]
    v2e = np.asarray(v2)[:, ::2, ::2].astype(np.float16)
    wf = np.stack([np.asarray(w_v1_hub), np.asarray(w_v2_hub),
                   np.asarray(w_v1_out), np.asarray(w_v2_out)]) \
        .astype(np.float16)  # [4, 512, 512]

    core_ids = list(range(N_CORES))
    in_maps = []
    for c in core_ids:
        bg, rg = divmod(c, RG)
        bsl = slice(bg * B_CORE, (bg + 1) * B_CORE)
        rsl = slice(rg * P, (rg + 1) * P)
        m = {"v1": np.ascontiguousarray(v1e[bsl, rsl, :].transpose(1, 0, 2)),
             "v2": np.ascontiguousarray(v2e[bsl, rsl, :].transpose(1, 0, 2)),
             "w4": np.ascontiguousarray(wf[:, rsl, :].transpose(1, 0, 2))}
        in_maps.append(m)

    res = run_bass_kernel_spmd(nc, in_maps, core_ids, **run_kwargs)

    hub = np.empty((B_FULL, TH, TW), np.float32)
    out = np.empty((B_FULL, TH, TW), np.float32)
    for c in core_ids:
        bg, rg = divmod(c, RG)
        buf = res.results[c]["ho"]  # [P, B_CORE, 2, TW] fp16
        for t, full in ((0, hub), (1, out)):
            full[bg * B_CORE:(bg + 1) * B_CORE,
                 rg * P:(rg + 1) * P, :] = \
                buf[:, :, t, :].transpose(1, 0, 2).astype(np.float32)
    kernel.last_results = res
    return (hub, out)
